# revision 1
# baseline (speedup 1.0000x reference)
"""Multi-head attention (B=2, S=2048, d_model=1024, 16 heads, dk=dv=64) on
8 Trainium2 NeuronCores.

Sharding: core = (batch, group-of-4-heads).  Each core projects q/k/v for its
4 heads (full sequence of its batch), runs softmax(q k^T) v without masking
(the harness mask is always all-True), applies its 256 rows of Wo, and returns
a partial [S, d_model] output.  The host sums the 4 partials per batch
(row-parallel Wo => host-side reduction instead of a device all-reduce).

Device layout notes:
  * Activations are fed pre-transposed ([d_model, S]) so d_model lands on
    SBUF partitions for the projection matmuls.
  * qh/kh are produced head-transposed ([dk, S], two heads stacked on the
    128 partitions); scores are computed transposed ([s_key, s_query]) with
    the two heads of a pair row-packed in the PE array (K=64 each).
  * vh carries an appended ones-column per head, so the attention@V matmul
    also produces the softmax denominators (row 64 of the PSUM result).
  * exp() runs on ScalarE straight out of PSUM in [128, 2048] ops.
"""

import numpy as np

import concourse.bass as bass
import concourse.mybir as mybir
import concourse.tile as tile
from concourse import bacc
from concourse.bass_utils import run_bass_kernel_spmd

P = 128
S = 2048
D = 1024
KT = D // P          # 8 k-tiles over d_model
NH = 4               # heads per core
DK = 64
NCORES = 8
F32 = mybir.dt.float32
BF16 = mybir.dt.bfloat16
AF = mybir.ActivationFunctionType

_CACHE: dict = {}
LAST_RESULTS = None  # test harness peeks at this for exec_time_ns


def _build_nc():
    nc = bacc.Bacc("TRN2", target_bir_lowering=False, num_devices=NCORES)

    qT = nc.dram_tensor("qT", [D, S], BF16, kind="ExternalInput").ap()
    kT = nc.dram_tensor("kT", [D, S], BF16, kind="ExternalInput").ap()
    vT = nc.dram_tensor("vT", [D, S], BF16, kind="ExternalInput").ap()
    wq = nc.dram_tensor("wq", [D, NH * DK], BF16, kind="ExternalInput").ap()
    wk = nc.dram_tensor("wk", [D, NH * DK], BF16, kind="ExternalInput").ap()
    wv = nc.dram_tensor("wv", [D, NH * DK], BF16, kind="ExternalInput").ap()
    wo = nc.dram_tensor("wo", [NH * DK, D], BF16, kind="ExternalInput").ap()
    out = nc.dram_tensor("outT", [D, S], F32, kind="ExternalOutput").ap()

    with tile.TileContext(nc) as tc:
        _build_body(nc, tc, qT, kT, vT, wq, wk, wv, wo, out)
    nc.compile()
    return nc


def _build_body(nc, tc, qT, kT, vT, wq, wk, wv, wo, out):
    from contextlib import ExitStack

    with ExitStack() as ctx:
        constp = ctx.enter_context(tc.tile_pool(name="const", bufs=1))

        # ---- persistent SBUF tensors -----------------------------------
        wq_s = constp.tile([P, KT, NH * DK], BF16)
        nc.sync.dma_start(wq_s, wq.rearrange("(kt p) n -> p kt n", p=P))
        wk_s = constp.tile([P, KT, NH * DK], BF16)
        nc.sync.dma_start(wk_s, wk.rearrange("(kt p) n -> p kt n", p=P))
        wv_s = constp.tile([P, KT, NH * DK], BF16)
        nc.sync.dma_start(wv_s, wv.rearrange("(kt p) n -> p kt n", p=P))
        wo_s = constp.tile([P, 2, D], BF16)
        nc.sync.dma_start(wo_s, wo.rearrange("(pair p) n -> p pair n", p=P))

        qhT = constp.tile([P, 2, S], BF16)   # [2 heads stacked, pair, S]
        khT = constp.tile([P, 2, S], BF16)
        # vh + ones column per head: [s_tile_part, s_tile, head, dv+1]
        vh_s = constp.tile([P, 16, NH, DK + 1], BF16)
        nc.any.memset(vh_s[:, :, :, DK], 1.0)
        # pair-stacked scaled attention output, transposed: [2*dv, S]
        avT = [constp.tile([P, S], BF16, name=f"avT{pr}") for pr in range(2)]

        # ---- q/k projections: qhT/khT = (Wq|Wk slice).T @ (q|k).T ------
        with tc.tile_pool(name="xfull", bufs=2) as xfp:
            with tc.tile_pool(name="pqk", bufs=1, space="PSUM") as pqk:
                for xdram, wsb, dst in ((qT, wq_s, qhT), (kT, wk_s, khT)):
                    xfull = xfp.tile([P, KT, S], BF16, tag="xf", name="xfull")
                    nc.sync.dma_start(
                        xfull, xdram.rearrange("(kt p) s -> p kt s", p=P)
                    )
                    psA = pqk.tile([P, S], F32, tag="projA", name="psA")
                    psB = pqk.tile([P, S], F32, tag="projB", name="psB")
                    # kt innermost: back-to-back accumulation into one PSUM
                    # region pipelines fill/drain (no bank cycling).
                    for pr, ps in enumerate((psA, psB)):
                        for c in range(4):
                            cs = slice(c * 512, (c + 1) * 512)
                            for kt in range(KT):
                                nc.tensor.matmul(
                                    ps[:, cs],
                                    wsb[:, kt, pr * P:(pr + 1) * P],
                                    xfull[:, kt, cs],
                                    start=(kt == 0),
                                    stop=(kt == KT - 1),
                                )
                    nc.vector.tensor_copy(dst[:, 0, :], psA)
                    nc.vector.tensor_copy(dst[:, 1, :], psB)

            # ---- v projection: vh = v @ Wv slice -----------------------
            # NB: each s-tile accumulates in its OWN psum bank — start=True
            # clears the whole bank, so sub-bank region packing is unsound.
            with tc.tile_pool(name="pv", bufs=4, space="PSUM") as pv:
                vfull = xfp.tile([P, KT, S], BF16, tag="xf", name="vfull")
                nc.sync.dma_start(vfull, vT.rearrange("(kt p) s -> p kt s", p=P))
                for st in range(16):
                    vp = pv.tile([P, NH * DK], F32, tag="vp", name="vp")
                    for kt in range(KT):
                        nc.tensor.matmul(
                            vp,
                            vfull[:, kt, st * P:(st + 1) * P],
                            wv_s[:, kt, :],
                            start=(kt == 0),
                            stop=(kt == KT - 1),
                        )
                    src = vp.rearrange("p (h d) -> p h d", h=NH)
                    nc.vector.tensor_copy(vh_s[:, st, :, 0:DK], src)

        # ---- attention: per head-pair, per query-half ------------------
        with (
            tc.tile_pool(name="pst", bufs=2, space="PSUM") as pst,
            tc.tile_pool(name="pav", bufs=1, space="PSUM") as pav,
            tc.tile_pool(name="attsb", bufs=2) as attsb,
        ):
            for pr in range(2):
                for ih in range(2):
                    i0 = ih * 1024
                    av_A = pav.tile([DK + 1, 1024], F32, tag="avA", name="av_A")
                    av_B = pav.tile([DK + 1, 1024], F32, tag="avB", name="av_B")
                    for j in range(16):
                        js = slice(j * P, (j + 1) * P)
                        # two [128, 1024] score tiles (i-quarters q0/q1), each
                        # holding head A in cols 0:512 and head B in 512:1024.
                        # bufs=2 on the pool => PE streams ahead of ScalarE.
                        stq = [
                            pst.tile([P, 1024], F32, tag="st", name="stq")
                            for _ in range(2)
                        ]
                        for c in range(2):  # A then B: disjoint row groups
                            ic = slice(i0 + c * 512, i0 + (c + 1) * 512)
                            nc.tensor.matmul(
                                stq[c][:, 0:512],
                                khT[0:DK, pr, js],
                                qhT[0:DK, pr, ic],
                                start=True, stop=True,
                            )
                            nc.tensor.matmul(
                                stq[c][:, 512:1024],
                                khT[DK:P, pr, js],
                                qhT[DK:P, pr, ic],
                                start=True, stop=True,
                            )
                        ptq = []
                        for c in range(2):
                            pt = attsb.tile([P, 1024], BF16, tag="pt", name="pt")
                            nc.scalar.activation(pt, stq[c], AF.Exp)
                            ptq.append(pt)
                        for c in range(2):  # vh_A stationary for both quarters
                            nc.tensor.matmul(
                                av_A[:, c * 512:(c + 1) * 512],
                                vh_s[:, j, 2 * pr, :],
                                ptq[c][:, 0:512],
                                start=(j == 0), stop=(j == 15),
                            )
                        for c in range(2):
                            nc.tensor.matmul(
                                av_B[:, c * 512:(c + 1) * 512],
                                vh_s[:, j, 2 * pr + 1, :],
                                ptq[c][:, 512:1024],
                                start=(j == 0), stop=(j == 15),
                            )
                    # softmax scale: divide by the ones-column sums (row DK).
                    # Copy PSUM->SBUF first so the psum banks free quickly;
                    # recip/bcast/mult run off the critical path on DVE/GpSimd.
                    for half, av in enumerate((av_A, av_B)):
                        av_sb = attsb.tile(
                            [DK + 1, 1024], F32, tag=f"avsb{half}", name="av_sb"
                        )
                        nc.vector.tensor_copy(av_sb, av)
                        rec = attsb.tile(
                            [1, 1024], F32, tag=f"rec{half}", name="rec"
                        )
                        nc.vector.reciprocal(rec, av_sb[DK:DK + 1, :])
                        bcs = attsb.tile(
                            [DK, 1024], F32, tag=f"bcs{half}", name="bcs"
                        )
                        nc.gpsimd.partition_broadcast(bcs, rec)
                        nc.vector.tensor_mul(
                            out=avT[pr][half * DK:(half + 1) * DK, i0:i0 + 1024],
                            in0=av_sb[0:DK, :],
                            in1=bcs,
                        )

        # ---- output projection, transposed: outT = Wo_slice.T @ av -----
        # Wo chunks are stationary (16 LDWEIGHTS total); avT streams.
        with (
            tc.tile_pool(name="po", bufs=2, space="PSUM") as po,
            tc.tile_pool(name="osb", bufs=2) as osb,
        ):
            for dc in range(8):
                ds_ = slice(dc * P, (dc + 1) * P)
                ops = po.tile([P, S], F32, tag="ops", name="ops")
                for c in range(4):
                    cs = slice(c * 512, (c + 1) * 512)
                    for pair in range(2):
                        nc.tensor.matmul(
                            ops[:, cs],
                            wo_s[:, pair, ds_],
                            avT[pair][:, cs],
                            start=(pair == 0), stop=(pair == 1),
                        )
                oto = osb.tile([P, S], F32, tag="oto", name="oto")
                nc.scalar.copy(oto, ops)
                nc.sync.dma_start(out[ds_, :], oto)


def kernel(q, k, v, mask, Wq, Wk, Wv, Wo, _trace=False, _tmpdir=None):
    """Full inputs in, full output out. mask is all-True by construction of
    the problem's input spec and is ignored (dense softmax)."""
    global LAST_RESULTS

    import ml_dtypes

    bf16 = ml_dtypes.bfloat16
    q = np.asarray(q, dtype=np.float32)
    k = np.asarray(k, dtype=np.float32)
    v = np.asarray(v, dtype=np.float32)
    Wq = np.asarray(Wq, dtype=bf16)
    Wk = np.asarray(Wk, dtype=bf16)
    Wv = np.asarray(Wv, dtype=bf16)
    Wo = np.asarray(Wo, dtype=bf16)
    B = q.shape[0]

    if "nc" not in _CACHE:
        _CACHE["nc"] = _build_nc()
    nc = _CACHE["nc"]

    qTb = [np.ascontiguousarray(q[b].T).astype(bf16) for b in range(B)]
    kTb = [np.ascontiguousarray(k[b].T).astype(bf16) for b in range(B)]
    vTb = [np.ascontiguousarray(v[b].T).astype(bf16) for b in range(B)]

    in_maps = []
    for core in range(NCORES):
        b, hg = core // 4, core % 4
        cs = slice(hg * NH * DK, (hg + 1) * NH * DK)
        in_maps.append({
            "qT": qTb[b],
            "kT": kTb[b],
            "vT": vTb[b],
            "wq": np.ascontiguousarray(Wq[:, cs]),
            "wk": np.ascontiguousarray(Wk[:, cs]),
            "wv": np.ascontiguousarray(Wv[:, cs]),
            "wo": np.ascontiguousarray(Wo[cs, :]),
        })

    res = run_bass_kernel_spmd(
        nc, in_maps, core_ids=list(range(NCORES)),
        trace=_trace, tmpdir=_tmpdir,
    )
    LAST_RESULTS = res

    fullT = np.zeros((B, D, S), dtype=np.float32)
    for core in range(NCORES):
        fullT[core // 4] += res.results[core]["outT"]
    return np.ascontiguousarray(fullT.transpose(0, 2, 1))



# revision 6
# speedup vs baseline: 1.0623x; 1.0623x over previous
"""Multi-head attention (B=2, S=2048, d_model=1024, 16 heads, dk=dv=64) on
8 Trainium2 NeuronCores.

Sharding: core = (batch, group-of-4-heads).  Each core projects q/k/v for its
4 heads (full sequence of its batch), runs softmax(q k^T) v without masking
(the harness mask is always all-True), applies its 256 rows of Wo, and returns
a partial [S, d_model] output.  The host sums the 4 partials per batch
(row-parallel Wo => host-side reduction instead of a device all-reduce).

v2 schedule (vs v1):
  * Inputs stream in 512-column chunks; the projections chase the DMA so the
    PE starts ~25us earlier.
  * Attention runs in 8 blocks of (head-pair, 512-query quarter).  Scores for
    the two heads of a pair go to different PE row-groups (tile_position
    auto-derived from base partitions 0/64) so they execute concurrently.
  * Output projection + output DMA are interleaved per query quarter instead
    of running as a serial tail.
  * Softmax denominators are staged into one [4, 512] tile via SBUF->SBUF DMA
    and inverted with a single reciprocal_approx_fast (the v1 per-row
    nc.vector.reciprocal was 8 x 6.5us on one lane).
  * PSUM budget: proj/oproj share 2 banks, scores 2x2, av 2x1 -> exactly 8.
"""

import numpy as np

import concourse.bass as bass
import concourse.mybir as mybir
import concourse.tile as tile
from concourse import bacc
from concourse.bass_utils import run_bass_kernel_spmd

P = 128
S = 2048
D = 1024
KT = D // P          # 8 k-tiles over d_model
NH = 4               # heads per core
DK = 64
NCORES = 8
CH = 512             # column chunk (keys/queries) for DMA + proj + attention
NCH = S // CH        # 4 chunks
F32 = mybir.dt.float32
BF16 = mybir.dt.bfloat16
AF = mybir.ActivationFunctionType

_CACHE: dict = {}
LAST_RESULTS = None  # test harness peeks at this for exec_time_ns


def _build_nc():
    nc = bacc.Bacc("TRN2", target_bir_lowering=False, num_devices=NCORES)

    qT = nc.dram_tensor("qT", [D, S], BF16, kind="ExternalInput").ap()
    kT = nc.dram_tensor("kT", [D, S], BF16, kind="ExternalInput").ap()
    vT = nc.dram_tensor("vT", [D, S], BF16, kind="ExternalInput").ap()
    wq = nc.dram_tensor("wq", [D, NH * DK], BF16, kind="ExternalInput").ap()
    wk = nc.dram_tensor("wk", [D, NH * DK], BF16, kind="ExternalInput").ap()
    wv = nc.dram_tensor("wv", [D, NH * DK], BF16, kind="ExternalInput").ap()
    wo = nc.dram_tensor("wo", [NH * DK, D], BF16, kind="ExternalInput").ap()
    out = nc.dram_tensor("outT", [D, S], F32, kind="ExternalOutput").ap()

    with tile.TileContext(nc) as tc:
        _build_body(nc, tc, qT, kT, vT, wq, wk, wv, wo, out)
    nc.compile()
    return nc


def _build_body(nc, tc, qT, kT, vT, wq, wk, wv, wo, out):
    from contextlib import ExitStack

    with ExitStack() as ctx:
        constp = ctx.enter_context(tc.tile_pool(name="const", bufs=1))

        # ---- persistent SBUF tensors -----------------------------------
        wq_s = constp.tile([P, KT, NH * DK], BF16)
        wk_s = constp.tile([P, KT, NH * DK], BF16)
        wv_s = constp.tile([P, KT, NH * DK], BF16)
        wo_s = constp.tile([P, 2, D], BF16)

        qhT = constp.tile([P, 2, S], BF16)   # [2 heads stacked, pair, S]
        khT = constp.tile([P, 2, S], BF16)
        # vh + ones column per head: [s_tile_part, s_tile, head, dv+1]
        vh_s = constp.tile([P, 16, NH, DK + 1], BF16)
        nc.any.memset(vh_s[:, :, :, DK], 1.0)
        # pair-stacked scaled attention output, transposed: [2*dv, S]
        avT = [constp.tile([P, S], BF16, name=f"avT{pr}") for pr in range(2)]
        # softmax denominators, all on partition 0 (gpsimd broadcast and the
        # batched reciprocal both need partition-0 sources): [1, head, ihq, CH]
        den_all = constp.tile([1, NH, NCH, CH], F32)
        rec_all = constp.tile([1, NH, NCH, CH], F32)

        # ---- input DMA, k first so the k-projection starts earliest ----
        nc.sync.dma_start(wk_s, wk.rearrange("(kt p) n -> p kt n", p=P))
        nc.sync.dma_start(wv_s, wv.rearrange("(kt p) n -> p kt n", p=P))
        nc.sync.dma_start(wq_s, wq.rearrange("(kt p) n -> p kt n", p=P))
        nc.sync.dma_start(wo_s, wo.rearrange("(pair p) n -> p pair n", p=P))

        xstream = ctx.enter_context(tc.tile_pool(name="xstream", bufs=6))
        xch: dict = {}
        for name, src in (("k", kT), ("v", vT), ("q", qT)):
            for c in range(NCH):
                cs = slice(c * CH, (c + 1) * CH)
                t = xstream.tile([P, KT, CH], BF16, tag="xs", name=f"x{name}{c}")
                nc.sync.dma_start(t, src[:, cs].rearrange("(kt p) s -> p kt s", p=P))
                xch[name, c] = t

        # pproj: shared 1-bank slots for q/k/v projections AND the output
        # projection (lifetimes interleave but tags share the 2 slots).
        pproj = ctx.enter_context(
            tc.tile_pool(name="pproj", bufs=2, space="PSUM"))
        pst = ctx.enter_context(tc.tile_pool(name="pst", bufs=2, space="PSUM"))
        pav = ctx.enter_context(tc.tile_pool(name="pav", bufs=1, space="PSUM"))
        attsb = ctx.enter_context(tc.tile_pool(name="attsb", bufs=3))
        osb = ctx.enter_context(tc.tile_pool(name="osb", bufs=2))

        # ---- q/k projections: chase the DMA chunks ---------------------
        def qk_proj(name, wsb, dst):
            for c in range(NCH):
                xc = xch[name, c]
                cs = slice(c * CH, (c + 1) * CH)
                for pr in range(2):
                    ps = pproj.tile([P, CH], F32, tag="pj", name="pj")
                    for kt in range(KT):
                        nc.tensor.matmul(
                            ps,
                            wsb[:, kt, pr * P:(pr + 1) * P],
                            xc[:, kt, :],
                            start=(kt == 0),
                            stop=(kt == KT - 1),
                        )
                    nc.vector.tensor_copy(dst[:, pr, cs], ps)

        def v_proj():
            # stationary = v chunk [128d, 128keys]; moving = Wv [128d, 256]
            for c in range(NCH):
                xc = xch["v", c]
                for sst in range(CH // P):
                    st = c * (CH // P) + sst
                    vp = pproj.tile([P, NH * DK], F32, tag="pj", name="pj")
                    for kt in range(KT):
                        nc.tensor.matmul(
                            vp,
                            xc[:, kt, sst * P:(sst + 1) * P],
                            wv_s[:, kt, :],
                            start=(kt == 0),
                            stop=(kt == KT - 1),
                        )
                    src = vp.rearrange("p (h d) -> p h d", h=NH)
                    nc.vector.tensor_copy(vh_s[:, st, :, 0:DK], src)

        qk_proj("k", wk_s, khT)
        v_proj()
        qk_proj("q", wq_s, qhT)

        # ---- attention + interleaved output projection -----------------
        for ihq in range(NCH):          # query quarter (512 queries)
            qs = slice(ihq * CH, (ihq + 1) * CH)
            for pr in range(2):         # head pair
                av_A = pav.tile([DK + 1, CH], F32, tag="avA", name="av_A")
                av_B = pav.tile([DK + 1, CH], F32, tag="avB", name="av_B")
                for j in range(16):
                    js = slice(j * P, (j + 1) * P)
                    stq = pst.tile([P, 2 * CH], F32, tag="st", name="stq")
                    # two heads -> different PE row groups, run concurrently
                    nc.tensor.matmul(
                        stq[:, 0:CH],
                        khT[0:DK, pr, js], qhT[0:DK, pr, qs],
                        start=True, stop=True,
                    )
                    nc.tensor.matmul(
                        stq[:, CH:2 * CH],
                        khT[DK:P, pr, js], qhT[DK:P, pr, qs],
                        start=True, stop=True,
                    )
                    pt = attsb.tile([P, 2 * CH], BF16, tag="pt", name="pt")
                    nc.scalar.activation(pt, stq, AF.Exp)
                    nc.tensor.matmul(
                        av_A,
                        vh_s[:, j, 2 * pr, :], pt[:, 0:CH],
                        start=(j == 0), stop=(j == 15),
                    )
                    nc.tensor.matmul(
                        av_B,
                        vh_s[:, j, 2 * pr + 1, :], pt[:, CH:2 * CH],
                        start=(j == 0), stop=(j == 15),
                    )
                for h, av in enumerate((av_A, av_B)):
                    r = 2 * pr + h
                    av_sb = attsb.tile([DK + 1, CH], F32, tag=f"avsb{r}",
                                       name="av_sb")
                    nc.vector.tensor_copy(av_sb, av)
                    # stage the denominator row (partition 64 -> partition 0)
                    nc.sync.dma_start(den_all[0:1, r, ihq, :],
                                      av_sb[DK:DK + 1, :])
                    xch[("avsb", pr, h)] = av_sb

            # one fast reciprocal for all 4 (pair, head) denominator rows
            nc.vector.reciprocal_approx_fast(
                rec_all[0:1, :, ihq, :], den_all[0:1, :, ihq, :])
            for pr in range(2):
                for h in range(2):
                    r = 2 * pr + h
                    av_sb = xch[("avsb", pr, h)]
                    bcs = attsb.tile([DK, CH], F32, tag="bcs", name="bcs")
                    nc.gpsimd.partition_broadcast(bcs, rec_all[0:1, r, ihq, :])
                    nc.vector.tensor_mul(
                        out=avT[pr][h * DK:(h + 1) * DK, qs],
                        in0=av_sb[0:DK, :],
                        in1=bcs,
                    )

            # output projection for this query quarter
            for dc in range(8):
                ds_ = slice(dc * P, (dc + 1) * P)
                ops = pproj.tile([P, CH], F32, tag="pj", name="pj")
                for pair in range(2):
                    nc.tensor.matmul(
                        ops,
                        wo_s[:, pair, ds_],
                        avT[pair][:, qs],
                        start=(pair == 0), stop=(pair == 1),
                    )
                oto = osb.tile([P, CH], F32, tag="oto", name="oto")
                if dc % 2 == 0:
                    nc.vector.tensor_copy(oto, ops)
                else:
                    nc.scalar.copy(oto, ops)
                nc.sync.dma_start(out[ds_, qs], oto)


def kernel(q, k, v, mask, Wq, Wk, Wv, Wo, _trace=False, _tmpdir=None):
    """Full inputs in, full output out. mask is all-True by construction of
    the problem's input spec and is ignored (dense softmax)."""
    global LAST_RESULTS

    import ml_dtypes

    bf16 = ml_dtypes.bfloat16
    q = np.asarray(q, dtype=np.float32)
    k = np.asarray(k, dtype=np.float32)
    v = np.asarray(v, dtype=np.float32)
    Wq = np.asarray(Wq, dtype=bf16)
    Wk = np.asarray(Wk, dtype=bf16)
    Wv = np.asarray(Wv, dtype=bf16)
    Wo = np.asarray(Wo, dtype=bf16)
    B = q.shape[0]

    if "nc" not in _CACHE:
        _CACHE["nc"] = _build_nc()
    nc = _CACHE["nc"]

    qTb = [np.ascontiguousarray(q[b].T).astype(bf16) for b in range(B)]
    kTb = [np.ascontiguousarray(k[b].T).astype(bf16) for b in range(B)]
    vTb = [np.ascontiguousarray(v[b].T).astype(bf16) for b in range(B)]

    in_maps = []
    for core in range(NCORES):
        b, hg = core // 4, core % 4
        cs = slice(hg * NH * DK, (hg + 1) * NH * DK)
        in_maps.append({
            "qT": qTb[b],
            "kT": kTb[b],
            "vT": vTb[b],
            "wq": np.ascontiguousarray(Wq[:, cs]),
            "wk": np.ascontiguousarray(Wk[:, cs]),
            "wv": np.ascontiguousarray(Wv[:, cs]),
            "wo": np.ascontiguousarray(Wo[cs, :]),
        })

    res = run_bass_kernel_spmd(
        nc, in_maps, core_ids=list(range(NCORES)),
        trace=_trace, tmpdir=_tmpdir,
    )
    LAST_RESULTS = res

    fullT = np.zeros((B, D, S), dtype=np.float32)
    for core in range(NCORES):
        fullT[core // 4] += res.results[core]["outT"]
    return np.ascontiguousarray(fullT.transpose(0, 2, 1))
